# revision 8
# baseline (speedup 1.0000x reference)
"""MoE (B=2,T=2048,D=768,E=8,H=1536,K=2) Trainium2 kernel.

Expert-parallel over the 8 NeuronCores: core e holds expert e's banks and
computes its weighted contribution w_e(t) * FFN_e(x_t) for all tokens; the
host sums the 8 contributions (tokens not routed to expert e get w_e=0).

All activations are kept feature-major (x^T [D, tok]) so that gate/up banks
[D,H] and down bank [H,D] are already in the stationary-operand (lhsT)
layout the PE wants — no transposes anywhere on device.

Routing runs on device: logits^T [8, tok] via matmul, exp (no max-sub
needed: |logits| < ~30), top-2 + normalized weights via partition-slice
reductions, then one [8x128]^T @ [8,tok] matmul against a one-hot matrix
to select-and-broadcast the expert's weight row across 128 partitions.
"""

import numpy as np

import concourse.bass as bass
import concourse.bass_isa as bass_isa
import concourse.mybir as mybir
import concourse.tile as tile
from concourse import bass_utils

# Problem shape (hardcoded per contract).
B, T, D, E, H = 2, 2048, 768, 8, 1536
NTOK = B * T            # 4096 tokens
TOK = 512               # tokens per block
NBLK = NTOK // TOK      # 8
DC = D // 128           # 6 chunks of the D (contraction) dim
HC = H // 128           # 12 chunks of the H dim
F32 = mybir.dt.float32


def _split_multiwaits(nc):
    """This walrus build only supports one sync-wait per instruction; move
    extra waits onto preceding NOPs on the same engine."""
    for fn in nc.m.functions:
        for bb in fn.blocks:
            out = []
            for ins in bb.instructions:
                si = ins.sync_info
                if si is not None and si.on_wait is not None and len(si.on_wait) > 1:
                    waits = list(si.on_wait)
                    for i, w in enumerate(waits[:-1]):
                        out.append(mybir.InstNoOp(
                            name=f"{ins.name}-sw{i}",
                            engine=ins.engine,
                            sync_info=mybir.SyncInfo(on_wait=[w], on_update=[]),
                        ))
                    si.on_wait = [waits[-1]]
                    ins.sync_info = si
                out.append(ins)
            bb.instructions = out


def build_nc():
    nc = bass.Bass()
    xT = nc.dram_tensor("xT", [D, NTOK], F32, kind="ExternalInput")
    gwT = nc.dram_tensor("gwT", [D, E], F32, kind="ExternalInput")
    gb = nc.dram_tensor("gb", [D, H], F32, kind="ExternalInput")
    ub = nc.dram_tensor("ub", [D, H], F32, kind="ExternalInput")
    db = nc.dram_tensor("db", [H, D], F32, kind="ExternalInput")
    esel = nc.dram_tensor("esel", [E, 128], F32, kind="ExternalInput")
    ident = nc.dram_tensor("ident", [128, 128], F32, kind="ExternalInput")
    yT = nc.dram_tensor("yT", [D, NTOK], F32, kind="ExternalOutput")

    xT_r = xT.rearrange("(c p) t -> p c t", p=128)   # [128, DC, NTOK]
    gwT_r = gwT.rearrange("(c p) e -> p c e", p=128)  # [128, DC, E]
    gb_r = gb.rearrange("(c p) h -> p c h", p=128)   # [128, DC, H]
    ub_r = ub.rearrange("(c p) h -> p c h", p=128)
    db_r = db.rearrange("(c p) d -> p c d", p=128)   # [128, HC, D]
    yT_r = yT.rearrange("(c p) t -> p c t", p=128)   # [128, DC, NTOK]

    with tile.TileContext(nc) as tc:
        with (
            tc.tile_pool(name="wts", bufs=1) as wts,
            tc.tile_pool(name="xp", bufs=2) as xp,
            tc.tile_pool(name="hp", bufs=14) as hp,
            tc.tile_pool(name="sap", bufs=2) as sap,
            tc.tile_pool(name="yp", bufs=3) as yp,
            tc.tile_pool(name="rp", bufs=1) as rp,
            tc.tile_pool(name="wsp", bufs=2) as wsp,
            tc.tile_pool(name="ps", bufs=8, space="PSUM") as ps,
        ):
            # Resident weights.
            gb_sb = wts.tile([128, DC, H], F32)
            nc.sync.dma_start(gb_sb[:], gb_r[:])
            ub_sb = wts.tile([128, DC, H], F32)
            nc.sync.dma_start(ub_sb[:], ub_r[:])
            db_sb = wts.tile([128, HC, D], F32)
            nc.sync.dma_start(db_sb[:], db_r[:])
            gwT_sb = wts.tile([128, DC, E], F32)
            nc.sync.dma_start(gwT_sb[:], gwT_r[:])
            esel_sb = wts.tile([E, 128], F32)
            nc.sync.dma_start(esel_sb[:], esel[:])
            id_sb = wts.tile([128, 128], F32)
            nc.sync.dma_start(id_sb[:], ident[:])

            for b in range(NBLK):
                blk = slice(b * TOK, (b + 1) * TOK)

                xb = xp.tile([128, DC, TOK], F32)
                nc.sync.dma_start(xb[:], xT_r[:, :, blk])

                # ---- routing (token-major): logits [128 tok, E] per tile ----
                # w8[e, t] = normalized top-2 gate weight, expert-major.
                w8_ps = ps.tile([E, TOK], F32, tag="ps")
                for tt in range(TOK // 128):
                    tsl = slice(tt * 128, (tt + 1) * 128)
                    lg_ps = ps.tile([128, E], F32, tag="ps")
                    for k in range(DC):
                        nc.tensor.matmul(lg_ps[:], xb[:, k, tsl],
                                         gwT_sb[:, k, :],
                                         start=(k == 0), stop=(k == DC - 1))
                    lg_sb = rp.tile([128, E], F32, tag="lg")
                    nc.vector.tensor_copy(lg_sb[:], lg_ps[:])
                    q = rp.tile([128, E], F32, tag="q")
                    nc.scalar.activation(q[:], lg_ps[:],
                                         mybir.ActivationFunctionType.Exp)
                    # top-2 selection on the raw logits
                    m1 = rp.tile([128, 1], F32, tag="m1")
                    nc.vector.reduce_max(m1[:], lg_sb[:],
                                         axis=mybir.AxisListType.X)
                    eq = rp.tile([128, E], F32, tag="eq")
                    nc.vector.tensor_tensor(eq[:], lg_sb[:],
                                            m1[:].to_broadcast((128, E)),
                                            op=mybir.AluOpType.is_equal)
                    nc.vector.tensor_scalar_mul(eq[:], eq[:], -1e9)
                    nc.vector.tensor_add(eq[:], eq[:], lg_sb[:])
                    m2 = rp.tile([128, 1], F32, tag="m2")
                    nc.vector.reduce_max(m2[:], eq[:], axis=mybir.AxisListType.X)
                    # mask = (logits >= m2) -> exactly the top-2
                    ge = rp.tile([128, E], F32, tag="ge")
                    nc.vector.tensor_tensor(ge[:], lg_sb[:],
                                            m2[:].to_broadcast((128, E)),
                                            op=mybir.AluOpType.is_ge)
                    nc.vector.tensor_mul(ge[:], ge[:], q[:])  # ge := q*mask
                    # denom = q_top1 + q_top2 + 1e-8 * sum(q)
                    den = rp.tile([128, 1], F32, tag="den")
                    nc.vector.reduce_sum(den[:], ge[:],
                                         axis=mybir.AxisListType.X)
                    zs = rp.tile([128, 1], F32, tag="zs")
                    nc.vector.reduce_sum(zs[:], q[:], axis=mybir.AxisListType.X)
                    nc.vector.tensor_scalar_mul(zs[:], zs[:], 1e-8)
                    nc.vector.tensor_add(den[:], den[:], zs[:])
                    nc.vector.reciprocal(den[:], den[:])
                    nc.vector.tensor_mul(ge[:], ge[:],
                                         den[:].to_broadcast((128, E)))
                    # transpose to expert-major via PE: w8 = ge.T @ I
                    nc.tensor.matmul(w8_ps[:, tsl], ge[:], id_sb[:],
                                     start=True, stop=True)
                w8_sb = rp.tile([E, TOK], F32, tag="w8")
                nc.vector.tensor_copy(w8_sb[:], w8_ps[:])
                # select expert row + broadcast to 128 partitions via matmul
                w_ps = ps.tile([128, TOK], F32, tag="ps")
                nc.tensor.matmul(w_ps[:], esel_sb[:], w8_sb[:],
                                 start=True, stop=True)
                wsb = wsp.tile([128, TOK], F32)
                nc.vector.tensor_copy(wsb[:], w_ps[:])

                # ---- FFN: h = silu(x@gb) * (x@ub) ----
                hts = []
                for ht in range(HC):
                    hsl = slice(ht * 128, (ht + 1) * 128)
                    a_ps = ps.tile([128, TOK], F32, tag="ps")
                    for k in range(DC):
                        nc.tensor.matmul(a_ps[:], gb_sb[:, k, hsl], xb[:, k, :],
                                         start=(k == 0), stop=(k == DC - 1))
                    u_ps = ps.tile([128, TOK], F32, tag="ps")
                    for k in range(DC):
                        nc.tensor.matmul(u_ps[:], ub_sb[:, k, hsl], xb[:, k, :],
                                         start=(k == 0), stop=(k == DC - 1))
                    sa = sap.tile([128, TOK], F32)
                    nc.scalar.activation(sa[:], a_ps[:],
                                         mybir.ActivationFunctionType.Silu)
                    hch = hp.tile([128, TOK], F32, tag="h")
                    nc.vector.tensor_mul(hch[:], sa[:], u_ps[:])
                    hts.append(hch)

                # ---- y^T = db^T @ h, scaled by routing weight ----
                for dt in range(DC):
                    dsl = slice(dt * 128, (dt + 1) * 128)
                    y_ps = ps.tile([128, TOK], F32, tag="ps")
                    for hk in range(HC):
                        nc.tensor.matmul(y_ps[:], db_sb[:, hk, dsl], hts[hk][:],
                                         start=(hk == 0), stop=(hk == HC - 1))
                    ysb = yp.tile([128, TOK], F32)
                    nc.vector.tensor_mul(ysb[:], y_ps[:], wsb[:])
                    nc.sync.dma_start(yT_r[:, dt, blk], ysb[:])

    _split_multiwaits(nc)
    return nc


_NC = None


def kernel(x, gate_w, gate_bank, up_bank, down_bank, _trace=False):
    global _NC
    if _NC is None:
        _NC = build_nc()
    nc = _NC

    xT = np.ascontiguousarray(x.reshape(NTOK, D).T).astype(np.float32)
    gwT = np.ascontiguousarray(gate_w.T).astype(np.float32)
    ident = np.eye(128, dtype=np.float32)
    in_maps = []
    for e in range(E):
        esel = np.zeros((E, 128), np.float32)
        esel[e, :] = 1.0
        in_maps.append({
            "xT": xT,
            "gwT": gwT,
            "gb": np.ascontiguousarray(gate_bank[e]).astype(np.float32),
            "ub": np.ascontiguousarray(up_bank[e]).astype(np.float32),
            "db": np.ascontiguousarray(down_bank[e]).astype(np.float32),
            "esel": esel,
            "ident": ident,
        })

    res = bass_utils.run_bass_kernel_spmd(
        nc, in_maps, core_ids=list(range(8)), trace=_trace)

    yT = res.results[0]["yT"].astype(np.float32)
    for e in range(1, E):
        yT = yT + res.results[e]["yT"].astype(np.float32)
    y = np.ascontiguousarray(yT.T).reshape(B, T, D).astype(np.float32)
    if _trace:
        return y, res
    return y


# revision 11
# speedup vs baseline: 9.5134x; 9.5134x over previous
"""MoE (B=2,T=2048,D=768,E=8,K=2,H=1536) Trainium2 kernel.

Sparse expert-parallel over the 8 NeuronCores: the host computes the gate
(softmax + top-2) in numpy, gathers the tokens routed to each expert, and
core e runs expert e's FFN only on its ~B*T*K/E gathered tokens. The
per-token gate weight is applied on device; the host scatter-adds the two
weighted expert outputs per token.

Activations stay feature-major (x^T [D, tok]) so gate/up banks [D,H] and
the down bank [H,D] are already in the stationary-operand (lhsT) layout the
PE wants — no transposes on device. The big GEMMs run in float32r (the PE's
single-pass fp32 mode, ~3.4x the 4-pass fp32 rate; per-GEMM rel err ~1.5e-4).
"""

import numpy as np

import concourse.bass as bass
import concourse.mybir as mybir
import concourse.tile as tile
from concourse import bass_utils

# Problem shape (hardcoded per contract).
B, T, D, E, H, KTOP = 2, 2048, 768, 8, 1536, 2
NTOK = B * T            # 4096 tokens
TOK = 512               # max tokens per block
DC = D // 128           # 6 chunks of the D (contraction) dim
HC = H // 128           # 12 chunks of the H dim
F32 = mybir.dt.float32
F32R = mybir.dt.float32r


def _install_axon_ntff_hook():
    """Best-effort: register the antenv.axon_hooks NTFF profile hook that the
    agent image lacks, so trace=True (or BASS_TRACE=1) can profile under axon.
    Never raises."""
    try:
        import sys, types, contextlib, ctypes  # noqa: PLC0415
        import antenv  # noqa: PLC0415
        if "antenv.axon_hooks" in sys.modules:
            return
        _HOOK = [None]
        mod = types.ModuleType("antenv.axon_hooks")
        mod.set_axon_ntff_profile_hook = lambda h: _HOOK.__setitem__(0, h)
        mod.get_axon_ntff_profile_hook = lambda: _HOOK[0]
        sys.modules["antenv.axon_hooks"] = mod
        antenv.axon_hooks = mod

        lib = ctypes.CDLL("/opt/axon/libaxon_pjrt.so")
        if not hasattr(lib, "axon_start_nrt_profile"):
            return
        lib.axon_start_nrt_profile.argtypes = [
            ctypes.POINTER(ctypes.c_int64), ctypes.c_size_t]
        lib.axon_start_nrt_profile.restype = ctypes.c_int64
        lib.axon_stop_nrt_profile.argtypes = [ctypes.c_char_p]
        lib.axon_stop_nrt_profile.restype = ctypes.c_int64

        @contextlib.contextmanager
        def _hook(output_dir, device_ids):
            import jax  # noqa: PLC0415
            jax.devices()
            if device_ids:
                ids = (ctypes.c_int64 * len(device_ids))(*device_ids)
                rc = lib.axon_start_nrt_profile(ids, len(device_ids))
            else:
                rc = lib.axon_start_nrt_profile(None, 0)
            if rc != 0:
                raise RuntimeError(f"axon_start_nrt_profile rc={rc}")
            try:
                yield
            finally:
                lib.axon_stop_nrt_profile(str(output_dir).encode())

        mod.set_axon_ntff_profile_hook(_hook)
    except Exception:
        pass


def _split_multiwaits(nc):
    """This walrus build only supports one sync-wait per instruction; move
    extra waits onto preceding NOPs on the same engine."""
    for fn in nc.m.functions:
        for bb in fn.blocks:
            out = []
            for ins in bb.instructions:
                si = ins.sync_info
                if si is not None and si.on_wait is not None and len(si.on_wait) > 1:
                    waits = list(si.on_wait)
                    for i, w in enumerate(waits[:-1]):
                        out.append(mybir.InstNoOp(
                            name=f"{ins.name}-sw{i}",
                            engine=ins.engine,
                            sync_info=mybir.SyncInfo(on_wait=[w], on_update=[]),
                        ))
                    si.on_wait = [waits[-1]]
                    ins.sync_info = si
                out.append(ins)
            bb.instructions = out
    return nc


def build_nc(npad):
    """Expert FFN on `npad` gathered tokens (feature-major, f32r GEMMs)."""
    blocks = []
    off = 0
    while off < npad:
        blocks.append((off, min(TOK, npad - off)))
        off += TOK

    nc = bass.Bass()
    xgT = nc.dram_tensor("xgT", [D, npad], F32R, kind="ExternalInput")
    gb = nc.dram_tensor("gb", [D, H], F32R, kind="ExternalInput")
    ub = nc.dram_tensor("ub", [D, H], F32R, kind="ExternalInput")
    db = nc.dram_tensor("db", [H, D], F32R, kind="ExternalInput")
    wrow = nc.dram_tensor("wrow", [1, npad], F32, kind="ExternalInput")
    ygT = nc.dram_tensor("ygT", [D, npad], F32, kind="ExternalOutput")

    xgT_r = xgT.rearrange("(c p) t -> p c t", p=128)   # [128, DC, npad]
    gb_r = gb.rearrange("(c p) h -> p c h", p=128)     # [128, DC, H]
    ub_r = ub.rearrange("(c p) h -> p c h", p=128)
    db_r = db.rearrange("(c p) d -> p c d", p=128)     # [128, HC, D]
    ygT_r = ygT.rearrange("(c p) t -> p c t", p=128)

    with tile.TileContext(nc) as tc:
        with (
            tc.tile_pool(name="wts", bufs=1) as wts,
            tc.tile_pool(name="xp", bufs=2) as xp,
            tc.tile_pool(name="hp", bufs=13) as hp,
            tc.tile_pool(name="sap", bufs=2) as sap,
            tc.tile_pool(name="yp", bufs=3) as yp,
            tc.tile_pool(name="wsp", bufs=2) as wsp,
            tc.tile_pool(name="ps", bufs=8, space="PSUM") as ps,
        ):
            # Resident expert banks (f32r straight from DRAM).
            gb_sb = wts.tile([128, DC, H], F32R)
            nc.sync.dma_start(gb_sb[:], gb_r[:])
            ub_sb = wts.tile([128, DC, H], F32R)
            nc.sync.dma_start(ub_sb[:], ub_r[:])
            db_sb = wts.tile([128, HC, D], F32R)
            nc.sync.dma_start(db_sb[:], db_r[:])
            wrow_sb = wts.tile([1, npad], F32)
            nc.sync.dma_start(wrow_sb[:], wrow[:])
            ones_sb = wts.tile([1, 128], F32)
            nc.vector.memset(ones_sb[:], 1.0)

            for off, tb in blocks:
                blk = slice(off, off + tb)

                xb = xp.tile([128, DC, TOK], F32R, tag="xb")
                nc.sync.dma_start(xb[:, :, 0:tb], xgT_r[:, :, blk])

                # Broadcast the per-token gate weight across 128 partitions:
                # W[p, t] = wrow[t] via ones[1,128].T @ wrow[1, tb].
                w_ps = ps.tile([128, TOK], F32, tag="ps")
                nc.tensor.matmul(w_ps[:, 0:tb], ones_sb[:], wrow_sb[:, blk],
                                 start=True, stop=True)
                wsb = wsp.tile([128, TOK], F32)
                nc.vector.tensor_copy(wsb[:, 0:tb], w_ps[:, 0:tb])

                # h = silu(x@gb) * (x@ub), feature-major [H, tb]
                hts = []
                for ht in range(HC):
                    hsl = slice(ht * 128, (ht + 1) * 128)
                    a_ps = ps.tile([128, TOK], F32, tag="ps")
                    for k in range(DC):
                        nc.tensor.matmul(a_ps[:, 0:tb], gb_sb[:, k, hsl],
                                         xb[:, k, 0:tb],
                                         start=(k == 0), stop=(k == DC - 1))
                    u_ps = ps.tile([128, TOK], F32, tag="ps")
                    for k in range(DC):
                        nc.tensor.matmul(u_ps[:, 0:tb], ub_sb[:, k, hsl],
                                         xb[:, k, 0:tb],
                                         start=(k == 0), stop=(k == DC - 1))
                    sa = sap.tile([128, TOK], F32)
                    nc.scalar.activation(sa[:, 0:tb], a_ps[:, 0:tb],
                                         mybir.ActivationFunctionType.Silu)
                    hch = hp.tile([128, TOK], F32R, tag="h")
                    nc.vector.tensor_mul(hch[:, 0:tb], sa[:, 0:tb],
                                         u_ps[:, 0:tb])
                    hts.append(hch)

                # y^T = db^T @ h, scaled by the gate weight
                for dt in range(DC):
                    dsl = slice(dt * 128, (dt + 1) * 128)
                    y_ps = ps.tile([128, TOK], F32, tag="ps")
                    for hk in range(HC):
                        nc.tensor.matmul(y_ps[:, 0:tb], db_sb[:, hk, dsl],
                                         hts[hk][:, 0:tb],
                                         start=(hk == 0), stop=(hk == HC - 1))
                    ysb = yp.tile([128, TOK], F32)
                    nc.vector.tensor_mul(ysb[:, 0:tb], y_ps[:, 0:tb],
                                         wsb[:, 0:tb])
                    nc.sync.dma_start(ygT_r[:, dt, blk], ysb[:, 0:tb])

    return _split_multiwaits(nc)


_NC_CACHE = {}


def _routing(x2d, gate_w):
    """Replicates the reference gate: softmax over E, top-2, renormalize."""
    logits = x2d @ gate_w.T                                  # [NTOK, E] f32
    lmax = logits.max(-1, keepdims=True)
    p = np.exp(logits - lmax)
    p = p / p.sum(-1, keepdims=True)
    idx = np.argsort(-p, axis=-1, kind="stable")[:, :KTOP]   # [NTOK, 2]
    sel = np.take_along_axis(p, idx, -1)
    w = sel / (sel.sum(-1, keepdims=True) + 1e-8)            # [NTOK, 2]
    return idx, w.astype(np.float32)


def kernel(x, gate_w, gate_bank, up_bank, down_bank, _trace=False):
    _install_axon_ntff_hook()
    x = np.asarray(x, dtype=np.float32)
    gate_w = np.asarray(gate_w, dtype=np.float32)
    x2d = np.ascontiguousarray(x.reshape(NTOK, D))

    idx, w = _routing(x2d, gate_w)

    # Token lists per expert.
    tok_idx = []
    tok_w = []
    for e in range(E):
        hit = (idx == e)                        # [NTOK, 2]
        rows = np.nonzero(hit.any(-1))[0]
        tok_idx.append(rows)
        tok_w.append(w[rows, np.argmax(hit[rows], axis=-1)])
    nmax = max(len(r) for r in tok_idx)
    npad = ((nmax + 127) // 128) * 128

    key = npad
    if key not in _NC_CACHE:
        _NC_CACHE[key] = build_nc(npad)
    nc = _NC_CACHE[key]

    in_maps = []
    for e in range(E):
        rows = tok_idx[e]
        xg = np.zeros((npad, D), np.float32)
        xg[: len(rows)] = x2d[rows]
        wr = np.zeros((1, npad), np.float32)
        wr[0, : len(rows)] = tok_w[e]
        in_maps.append({
            "xgT": np.ascontiguousarray(xg.T),
            "gb": np.ascontiguousarray(gate_bank[e], dtype=np.float32),
            "ub": np.ascontiguousarray(up_bank[e], dtype=np.float32),
            "db": np.ascontiguousarray(down_bank[e], dtype=np.float32),
            "wrow": wr,
        })

    res = bass_utils.run_bass_kernel_spmd(
        nc, in_maps, core_ids=list(range(8)), trace=_trace)

    y = np.zeros((NTOK, D), np.float32)
    for e in range(E):
        rows = tok_idx[e]
        y[rows] += res.results[e]["ygT"][:, : len(rows)].T
    y = y.reshape(B, T, D)
    if _trace:
        return y, res
    return y


# revision 15
# speedup vs baseline: 10.4594x; 1.0994x over previous
"""MoE (B=2,T=2048,D=768,E=8,K=2,H=1536) Trainium2 kernel.

Sparse expert-parallel over the 8 NeuronCores: the host computes the gate
(softmax + top-2) in numpy, gathers the tokens routed to each expert, and
core e runs expert e's FFN only on its ~B*T*K/E gathered tokens. The
per-token gate weight is applied on device; the host scatter-adds the two
weighted expert outputs per token.

Activations stay feature-major (x^T [D, tok]) so gate/up banks [D,H] and
the down bank [H,D] are already in the stationary-operand (lhsT) layout the
PE wants — no transposes on device. The big GEMMs run in float32r (the PE's
single-pass fp32 mode, ~3.4x the 4-pass fp32 rate; per-GEMM rel err ~1.5e-4).
"""

import numpy as np

import concourse.bass as bass
import concourse.mybir as mybir
import concourse.tile as tile
from concourse import bass_utils

# Problem shape (hardcoded per contract).
B, T, D, E, H, KTOP = 2, 2048, 768, 8, 1536, 2
NTOK = B * T            # 4096 tokens
TOK = 512               # max tokens per block
DC = D // 128           # 6 chunks of the D (contraction) dim
HC = H // 128           # 12 chunks of the H dim
F32 = mybir.dt.float32
F32R = mybir.dt.float32r


def _install_axon_ntff_hook():
    """Best-effort: register the antenv.axon_hooks NTFF profile hook that the
    agent image lacks, so trace=True (or BASS_TRACE=1) can profile under axon.
    Never raises."""
    try:
        import sys, types, contextlib, ctypes  # noqa: PLC0415
        import antenv  # noqa: PLC0415
        if "antenv.axon_hooks" in sys.modules:
            return
        _HOOK = [None]
        mod = types.ModuleType("antenv.axon_hooks")
        mod.set_axon_ntff_profile_hook = lambda h: _HOOK.__setitem__(0, h)
        mod.get_axon_ntff_profile_hook = lambda: _HOOK[0]
        sys.modules["antenv.axon_hooks"] = mod
        antenv.axon_hooks = mod

        lib = ctypes.CDLL("/opt/axon/libaxon_pjrt.so")
        if not hasattr(lib, "axon_start_nrt_profile"):
            return
        lib.axon_start_nrt_profile.argtypes = [
            ctypes.POINTER(ctypes.c_int64), ctypes.c_size_t]
        lib.axon_start_nrt_profile.restype = ctypes.c_int64
        lib.axon_stop_nrt_profile.argtypes = [ctypes.c_char_p]
        lib.axon_stop_nrt_profile.restype = ctypes.c_int64

        @contextlib.contextmanager
        def _hook(output_dir, device_ids):
            import jax  # noqa: PLC0415
            jax.devices()
            if device_ids:
                ids = (ctypes.c_int64 * len(device_ids))(*device_ids)
                rc = lib.axon_start_nrt_profile(ids, len(device_ids))
            else:
                rc = lib.axon_start_nrt_profile(None, 0)
            if rc != 0:
                raise RuntimeError(f"axon_start_nrt_profile rc={rc}")
            try:
                yield
            finally:
                lib.axon_stop_nrt_profile(str(output_dir).encode())

        mod.set_axon_ntff_profile_hook(_hook)
    except Exception:
        pass


def _split_multiwaits(nc):
    """This walrus build only supports one sync-wait per instruction; move
    extra waits onto preceding NOPs on the same engine."""
    for fn in nc.m.functions:
        for bb in fn.blocks:
            out = []
            for ins in bb.instructions:
                si = ins.sync_info
                if si is not None and si.on_wait is not None and len(si.on_wait) > 1:
                    waits = list(si.on_wait)
                    for i, w in enumerate(waits[:-1]):
                        out.append(mybir.InstNoOp(
                            name=f"{ins.name}-sw{i}",
                            engine=ins.engine,
                            sync_info=mybir.SyncInfo(on_wait=[w], on_update=[]),
                        ))
                    si.on_wait = [waits[-1]]
                    ins.sync_info = si
                out.append(ins)
            bb.instructions = out
    return nc


def build_nc(npad):
    """Expert FFN on `npad` gathered tokens (feature-major, f32r GEMMs)."""
    # Equal-ish blocks of at most TOK tokens (multiples of 128): balanced
    # blocks beat [512, 512, remainder] because per-block matmul count is
    # fixed while per-matmul cost scales with N.
    ntile = npad // 128
    nblk = -(-ntile // (TOK // 128))
    sizes = [(ntile // nblk + (1 if i < ntile % nblk else 0)) * 128
             for i in range(nblk)]
    blocks = []
    off = 0
    for s in sizes:
        blocks.append((off, s))
        off += s

    nc = bass.Bass()
    xgT = nc.dram_tensor("xgT", [D, npad], F32R, kind="ExternalInput")
    gb = nc.dram_tensor("gb", [D, H], F32R, kind="ExternalInput")
    ub = nc.dram_tensor("ub", [D, H], F32R, kind="ExternalInput")
    db = nc.dram_tensor("db", [H, D], F32R, kind="ExternalInput")
    wrow = nc.dram_tensor("wrow", [1, npad], F32, kind="ExternalInput")
    ygT = nc.dram_tensor("ygT", [D, npad], F32, kind="ExternalOutput")

    xgT_r = xgT.rearrange("(c p) t -> p c t", p=128)   # [128, DC, npad]
    gb_r = gb.rearrange("(c p) h -> p c h", p=128)     # [128, DC, H]
    ub_r = ub.rearrange("(c p) h -> p c h", p=128)
    db_r = db.rearrange("(c p) d -> p c d", p=128)     # [128, HC, D]
    ygT_r = ygT.rearrange("(c p) t -> p c t", p=128)

    with tile.TileContext(nc) as tc:
        with (
            tc.tile_pool(name="wts", bufs=1) as wts,
            tc.tile_pool(name="xp", bufs=2) as xp,
            tc.tile_pool(name="hp", bufs=13) as hp,
            tc.tile_pool(name="sap", bufs=2) as sap,
            tc.tile_pool(name="yp", bufs=3) as yp,
            tc.tile_pool(name="wsp", bufs=2) as wsp,
            tc.tile_pool(name="ps", bufs=8, space="PSUM") as ps,
        ):
            # Resident expert banks (f32r straight from DRAM), chunked and on
            # the ACT HWDGE ring (nc.scalar) so the x-block / output DMAs on
            # the SP ring (nc.sync) don't queue behind 14 MB of weights.
            # gb/ub interleave per D-chunk so GEMM1's k-loop streams as the
            # chunks land; db follows (needed only when GEMM2 starts).
            wrow_sb = wts.tile([1, npad], F32)
            nc.scalar.dma_start(wrow_sb[:], wrow[:])
            gb_k = [wts.tile([128, H], F32R, tag=f"gb{k}", name=f"gb{k}")
                    for k in range(DC)]
            ub_k = [wts.tile([128, H], F32R, tag=f"ub{k}", name=f"ub{k}")
                    for k in range(DC)]
            for k in range(DC):
                nc.scalar.dma_start(gb_k[k][:], gb_r[:, k, :])
                nc.scalar.dma_start(ub_k[k][:], ub_r[:, k, :])
            db_k = [wts.tile([128, D], F32R, tag=f"db{k}", name=f"db{k}")
                    for k in range(HC)]
            for k in range(HC):
                nc.scalar.dma_start(db_k[k][:], db_r[:, k, :])
            ones_sb = wts.tile([1, 128], F32)
            nc.vector.memset(ones_sb[:], 1.0)

            for off, tb in blocks:
                blk = slice(off, off + tb)

                xb = xp.tile([128, DC, TOK], F32R, tag="xb")
                nc.sync.dma_start(xb[:, :, 0:tb], xgT_r[:, :, blk])

                # Broadcast the per-token gate weight across 128 partitions:
                # W[p, t] = wrow[t] via ones[1,128].T @ wrow[1, tb].
                w_ps = ps.tile([128, TOK], F32, tag="ps")
                nc.tensor.matmul(w_ps[:, 0:tb], ones_sb[:], wrow_sb[:, blk],
                                 start=True, stop=True)
                wsb = wsp.tile([128, TOK], F32)
                nc.vector.tensor_copy(wsb[:, 0:tb], w_ps[:, 0:tb])

                # h = silu(x@gb) * (x@ub), feature-major [H, tb]
                hts = []
                for ht in range(HC):
                    hsl = slice(ht * 128, (ht + 1) * 128)
                    a_ps = ps.tile([128, TOK], F32, tag="ps")
                    for k in range(DC):
                        nc.tensor.matmul(a_ps[:, 0:tb], gb_k[k][:, hsl],
                                         xb[:, k, 0:tb],
                                         start=(k == 0), stop=(k == DC - 1))
                    u_ps = ps.tile([128, TOK], F32, tag="ps")
                    for k in range(DC):
                        nc.tensor.matmul(u_ps[:, 0:tb], ub_k[k][:, hsl],
                                         xb[:, k, 0:tb],
                                         start=(k == 0), stop=(k == DC - 1))
                    sa = sap.tile([128, TOK], F32)
                    nc.scalar.activation(sa[:, 0:tb], a_ps[:, 0:tb],
                                         mybir.ActivationFunctionType.Silu)
                    hch = hp.tile([128, TOK], F32R, tag="h")
                    nc.vector.tensor_mul(hch[:, 0:tb], sa[:, 0:tb],
                                         u_ps[:, 0:tb])
                    hts.append(hch)

                # y^T = db^T @ h, scaled by the gate weight
                for dt in range(DC):
                    dsl = slice(dt * 128, (dt + 1) * 128)
                    y_ps = ps.tile([128, TOK], F32, tag="ps")
                    for hk in range(HC):
                        nc.tensor.matmul(y_ps[:, 0:tb], db_k[hk][:, dsl],
                                         hts[hk][:, 0:tb],
                                         start=(hk == 0), stop=(hk == HC - 1))
                    ysb = yp.tile([128, TOK], F32)
                    nc.vector.tensor_mul(ysb[:, 0:tb], y_ps[:, 0:tb],
                                         wsb[:, 0:tb])
                    nc.sync.dma_start(ygT_r[:, dt, blk], ysb[:, 0:tb])

    return _split_multiwaits(nc)


_NC_CACHE = {}


def _routing(x2d, gate_w):
    """Replicates the reference gate: softmax over E, top-2, renormalize."""
    logits = x2d @ gate_w.T                                  # [NTOK, E] f32
    lmax = logits.max(-1, keepdims=True)
    p = np.exp(logits - lmax)
    p = p / p.sum(-1, keepdims=True)
    idx = np.argsort(-p, axis=-1, kind="stable")[:, :KTOP]   # [NTOK, 2]
    sel = np.take_along_axis(p, idx, -1)
    w = sel / (sel.sum(-1, keepdims=True) + 1e-8)            # [NTOK, 2]
    return idx, w.astype(np.float32)


def kernel(x, gate_w, gate_bank, up_bank, down_bank, _trace=False):
    _install_axon_ntff_hook()
    x = np.asarray(x, dtype=np.float32)
    gate_w = np.asarray(gate_w, dtype=np.float32)
    x2d = np.ascontiguousarray(x.reshape(NTOK, D))

    idx, w = _routing(x2d, gate_w)

    # Token lists per expert.
    tok_idx = []
    tok_w = []
    for e in range(E):
        hit = (idx == e)                        # [NTOK, 2]
        rows = np.nonzero(hit.any(-1))[0]
        tok_idx.append(rows)
        tok_w.append(w[rows, np.argmax(hit[rows], axis=-1)])
    nmax = max(len(r) for r in tok_idx)
    npad = ((nmax + 127) // 128) * 128

    key = npad
    if key not in _NC_CACHE:
        _NC_CACHE[key] = build_nc(npad)
    nc = _NC_CACHE[key]

    in_maps = []
    for e in range(E):
        rows = tok_idx[e]
        xg = np.zeros((npad, D), np.float32)
        xg[: len(rows)] = x2d[rows]
        wr = np.zeros((1, npad), np.float32)
        wr[0, : len(rows)] = tok_w[e]
        in_maps.append({
            "xgT": np.ascontiguousarray(xg.T),
            "gb": np.ascontiguousarray(gate_bank[e], dtype=np.float32),
            "ub": np.ascontiguousarray(up_bank[e], dtype=np.float32),
            "db": np.ascontiguousarray(down_bank[e], dtype=np.float32),
            "wrow": wr,
        })

    res = bass_utils.run_bass_kernel_spmd(
        nc, in_maps, core_ids=list(range(8)), trace=_trace)

    y = np.zeros((NTOK, D), np.float32)
    for e in range(E):
        rows = tok_idx[e]
        y[rows] += res.results[e]["ygT"][:, : len(rows)].T
    y = y.reshape(B, T, D)
    if _trace:
        return y, res
    return y
